# revision 8
# baseline (speedup 1.0000x reference)
"""DitLinearTemporalSelfAttention on 8 TRN2 NeuronCores (Bass/Tile).

Sharding: token-parallel. Core c handles batch b=c//2, token half c%2
(2048 tokens, full D=1024). The temporal-softmax/context reduction over
T=4096 spans two cores per batch -> pairwise AllReduce [[0,1],[2,3],...]
of the tiny per-batch [H,dh,dh+1] context+ksum buffer (266 KB).

Math (per core, tokens t in its slice):
  xn   = LN(x) bf16, with norm_g folded into weights host-side
  kT: fp8 DoubleRow projection (xnT fp8 x Wk fp8*64), exp(psum/64) -> ke bf16
  vv: bf16 projection, copied into va with ones column
  ctx_unnorm[h,d,l] = sum_t expk[t,d] * v[t,l];  ksum via ones-column of v
  (pairwise AllReduce) -> ctx = ctx_unnorm / ksum; append ones column
  qT: fp8 DoubleRow projection out[j,t], exp(psum/64) -> qT bf16
  y[t,l]+qden[t] = expqT.T @ ctx_aug per head (bf16); y /= qden
  h = silu(LN(y)*scale2 + shift2);  out = x + h @ (gate*out_W)
"""

import numpy as np

import concourse.bass as bass
import concourse.bacc as bacc
import concourse.mybir as mybir
import concourse.tile as tile
from concourse import masks
from concourse.bass_utils import run_bass_kernel_spmd

B, T, D, H, DH = 4, 4096, 1024, 16, 64
NCORES = 8
TL = T // 2          # tokens per core
NT = TL // 128       # 16 token tiles
ND = D // 128        # 8 d-chunks
EPS = 1e-5
WS = 64.0            # fp8 weight pre-scale (pow2; unscaled via exp scale)
FP32 = mybir.dt.float32
FP32R = mybir.dt.float32r
BF16 = mybir.dt.bfloat16
FP8 = mybir.dt.float8e4
I32 = mybir.dt.int32
DR = mybir.MatmulPerfMode.DoubleRow
QUAKE = 0x5F3759DF

_CACHE = {}
USE_COLLECTIVE = True
TRUNC = None  # 1=const/emb, 2=+stage0, 3=+stage1+coll, 4=+qproj, 5=+normalize, None=full
USE_NATIVE_SILU = True  # sim lacks Silu; simtest flips this


def r32(ap):
    return ap.bitcast(FP32R)


def _legalize_waits(nc, cap=2, escap=2):
    """Split >cap semaphore waits off any instruction into EventSemaphore
    instructions placed immediately before it on the same engine (walrus
    codegen structs hold only a few sync-wait slots)."""
    n = 0
    for bb in nc.main_func.blocks:
        out = []
        changed = False
        for ins in bb.instructions:
            si = ins.sync_info
            ty = type(ins).__name__
            icap = 1 if ty == "InstDMACopy" else cap
            if (si is not None and si.on_wait is not None
                    and len(si.on_wait) > icap
                    and ty not in ("InstDrain", "InstEventSemaphore")):
                waits = list(si.on_wait)
                keep, extra = waits[:icap], waits[icap:]
                while extra:
                    chunk, extra = extra[:escap], extra[escap:]
                    n += 1
                    es = mybir.InstEventSemaphore(
                        name=f"I-wsplit-{n}", engine=ins.engine,
                        sync_info=mybir.SyncInfo(on_wait=list(chunk),
                                                 on_update=[]))
                    out.append(es)
                ins.sync_info = mybir.SyncInfo(
                    on_wait=keep, on_update=list(si.on_update or []))
                changed = True
            out.append(ins)
        if changed:
            bb.instructions = out
    return n


def build(has_cq, has_ck, has_cv, has_co, has_cemb):
    from contextlib import ExitStack

    nc = bacc.Bacc("TRN2", target_bir_lowering=False, debug=False,
                   num_devices=NCORES)

    x_d = nc.dram_tensor("x", [TL, D], FP32, kind="ExternalInput")
    wk_d = nc.dram_tensor("wk8", [D, D], FP8, kind="ExternalInput")
    wv_d = nc.dram_tensor("wv", [D, D], BF16, kind="ExternalInput")
    wq_d = nc.dram_tensor("wq8", [D, D], FP8, kind="ExternalInput")
    wo_d = nc.dram_tensor("wo", [D, D], BF16, kind="ExternalInput")
    embw_d = nc.dram_tensor("embw", [D, 2 * D], BF16, kind="ExternalInput")
    embt_d = nc.dram_tensor("embt", [D], FP32, kind="ExternalInput")
    gsn_d = nc.dram_tensor("gsn", [2, D], FP32R, kind="ExternalInput")
    cemb_d = nc.dram_tensor("cemb", [2 * D], FP32R, kind="ExternalInput") if has_cemb else None
    cq_d = nc.dram_tensor("cq", [D], FP32R, kind="ExternalInput") if has_cq else None
    ck_d = nc.dram_tensor("ck", [D], FP32R, kind="ExternalInput") if has_ck else None
    cv_d = nc.dram_tensor("cv", [D], FP32R, kind="ExternalInput") if has_cv else None
    co_d = nc.dram_tensor("co", [D], FP32R, kind="ExternalInput") if has_co else None
    out_d = nc.dram_tensor("out", [TL, D], FP32, kind="ExternalOutput")

    def _emit(tc, es):
        constp = es.enter_context(tc.tile_pool(name="const", bufs=1))
        xio = es.enter_context(tc.tile_pool(name="xio", bufs=2))
        statp = es.enter_context(tc.tile_pool(name="stat", bufs=4))
        dramp = es.enter_context(tc.tile_pool(name="dram", bufs=1, space="DRAM"))
        tp = es.enter_context(tc.tile_pool(name="tp", bufs=2, space="PSUM"))
        pp = es.enter_context(tc.tile_pool(name="pp", bufs=6, space="PSUM"))

        # ---------------- constants ----------------
        ident = constp.tile([128, 128], FP32)
        masks.make_identity(nc, ident[:])
        identb = constp.tile([128, 128], BF16)
        nc.vector.tensor_copy(identb[:], ident[:])
        ones_row32 = constp.tile([1, 512], FP32)
        nc.vector.memset(ones_row32[:], 1.0)
        ones_row = constp.tile([1, 512], FP32R)
        nc.vector.tensor_copy(ones_row[:], ones_row32[:])
        kq_i = constp.tile([128, 1], I32)
        nc.vector.memset(kq_i[:], QUAKE)
        eps_col = constp.tile([128, 1], FP32)
        nc.vector.memset(eps_col[:], EPS)

        # rows: sng/snb via ONE dma; bias rows when present
        gsn = constp.tile([1, 2 * D], FP32R)
        nc.sync.dma_start(out=gsn[:], in_=gsn_d[:].rearrange("a b -> (a b)").unsqueeze(0))
        sng_row = gsn[:, 0:D]
        snb_row = gsn[:, D:2 * D]

        def load_row(pool, dram_ap, n):
            t_ = pool.tile([1, n], FP32R, tag=dram_ap.tensor.name)
            nc.sync.dma_start(out=t_[:], in_=dram_ap.unsqueeze(0))
            return t_

        cemb_row = load_row(constp, cemb_d.ap(), 2 * D) if has_cemb else None
        cq_row = load_row(constp, cq_d.ap(), D) if has_cq else None
        ck_row = load_row(constp, ck_d.ap(), D) if has_ck else None
        cv_row = load_row(constp, cv_d.ap(), D) if has_cv else None
        co_row = load_row(constp, co_d.ap(), D) if has_co else None

        def finish_tail(consumes):
            for t in range(NT):
                xt2 = xio.tile([128, D], FP32, tag="xin")
                nc.gpsimd.dma_start(out=xt2[:], in_=x_d[t * 128:(t + 1) * 128, :])
                fin = xio.tile([128, D], FP32, tag="fin")
                nc.vector.tensor_copy(fin[:], xt2[:])
                if t == 0:
                    for i, ap in enumerate(consumes):
                        nc.vector.tensor_copy(fin[0:1, i * 4:i * 4 + 4], ap)
                nc.gpsimd.dma_start(out=out_d[t * 128:(t + 1) * 128, :], in_=fin[:])

        # xnT opens BEFORE setup transients so it never reuses their zone
        es_xnt = ExitStack()
        xntp = es_xnt.enter_context(tc.tile_pool(name="xnT", bufs=1))
        xnt = xntp.tile([128, ND * TL], BF16)
        xnt8 = xntp.tile([128, ND * TL], FP8)

        es_rows = ExitStack()
        rowsp = es_rows.enter_context(tc.tile_pool(name="rows", bufs=1))

        # embt host-permuted: one DMA fills [128, 8], (p, c) = emb[c*128+p]
        embt_sb = rowsp.tile([128, ND], FP32)
        nc.sync.dma_start(out=embt_sb[:], in_=embt_d[:].rearrange(
            "(p c) -> p c", c=ND))
        silu_e = rowsp.tile([128, ND], FP32)
        if USE_NATIVE_SILU:
            nc.scalar.activation(silu_e[:], embt_sb[:],
                                 mybir.ActivationFunctionType.Silu)
        else:
            nc.scalar.activation(silu_e[:], embt_sb[:],
                                 mybir.ActivationFunctionType.Sigmoid)
            nc.vector.tensor_tensor(silu_e[:], silu_e[:], embt_sb[:],
                                    mybir.AluOpType.mult)

        # ------------- emb MLP: bf16 matvec on PE (one embw DMA) -------------
        silu_eb = rowsp.tile([128, 2 * ND], BF16)
        nc.vector.tensor_copy(
            silu_eb[:].rearrange("p (c two) -> p c two", two=2)[:, :, 0:1],
            silu_e[:].unsqueeze(2))
        embw = rowsp.tile([128, ND * 2 * D], BF16)  # d-chunk dc at cols dc*2048
        nc.sync.dma_start(
            out=embw[:].rearrange("p (dc c) -> p dc c", c=2 * D),
            in_=embw_d[:].rearrange("(dc p) c -> p dc c", p=128))
        emb_sel = rowsp.tile([1, 2 * D], FP32R)
        for nch in range(4):
            epn = pp.tile([1, 512], FP32, tag="pp")
            for dc in range(ND):
                nc.tensor.matmul(epn[:],
                                 silu_eb[:, 2 * dc:2 * dc + 1],
                                 embw[:, dc * 2048 + nch * 512:dc * 2048 + (nch + 1) * 512],
                                 start=(dc == 0), stop=(dc == ND - 1))
            nc.vector.tensor_copy(emb_sel[:, nch * 512:(nch + 1) * 512], epn[:])
        if has_cemb:
            nc.vector.tensor_tensor(emb_sel[:], emb_sel[:], cemb_row[:],
                                    mybir.AluOpType.add)
        # broadcast emb_sel + sng/snb rows to all partitions
        emb_sel_b = rowsp.tile([128, 2 * D], FP32)
        for nch in range(4):
            bp = pp.tile([128, 512], FP32, tag="pp")
            nc.tensor.matmul(bp[:], ones_row[:, 0:128],
                             emb_sel[:, nch * 512:(nch + 1) * 512])
            nc.vector.tensor_copy(emb_sel_b[:, nch * 512:(nch + 1) * 512], bp[:])

        def bcast(row, name):
            out = constp.tile([128, D], FP32, tag=f"bc_{name}")
            for nh in range(2):
                bp = pp.tile([128, 512], FP32, tag="pp")
                nc.tensor.matmul(bp[:], ones_row[:, 0:128],
                                 row[:, nh * 512:(nh + 1) * 512])
                nc.vector.tensor_copy(out[:, nh * 512:(nh + 1) * 512], bp[:])
            return out

        sng_b = bcast(sng_row, "sng")
        snb_b = bcast(snb_row, "snb")
        # scale2 = sng*(1+scale); shift2 = snb*(1+scale) + shift
        t1_b = rowsp.tile([128, D], FP32)
        nc.vector.tensor_scalar(t1_b[:], emb_sel_b[:, 0:D], 1.0, None,
                                mybir.AluOpType.add)
        s2_b = constp.tile([128, D], FP32)
        nc.vector.tensor_tensor(s2_b[:], t1_b[:], sng_b[:],
                                mybir.AluOpType.mult)
        sh2_b = constp.tile([128, D], FP32)
        nc.vector.tensor_tensor(sh2_b[:], t1_b[:], snb_b[:],
                                mybir.AluOpType.mult)
        nc.vector.tensor_tensor(sh2_b[:], sh2_b[:], emb_sel_b[:, D:2 * D],
                                mybir.AluOpType.add)

        if TRUNC == 1:
            finish_tail([s2_b[0:1, 0:4], sh2_b[0:1, 0:4]])
            es_rows.close()
            es_xnt.close()
            return

        # ---------------- stage 0 emitter: load x, LN, transpose ----------------
        def em_s0(t):
            xt = xio.tile([128, D], FP32, tag="xin")
            nc.sync.dma_start(out=xt[:], in_=x_d[t * 128:(t + 1) * 128, :])
            st6 = statp.tile([128, 2, 6], FP32, tag="st6")
            nc.vector.bn_stats(st6[:, 0, :], xt[:, 0:512])
            nc.vector.bn_stats(st6[:, 1, :], xt[:, 512:1024])
            agg = statp.tile([128, 2], FP32, tag="agg")
            nc.vector.bn_aggr(agg[:], st6[:])
            # rstd = exp(-0.5*ln(var+eps)): Ln/Exp live in one act table set,
            # so no table reload against the k/q Exp epilogues (Sqrt doesn't).
            lnv = statp.tile([128, 1], FP32, tag="lnv")
            nc.scalar.activation(lnv[:], agg[:, 1:2],
                                 mybir.ActivationFunctionType.Ln,
                                 bias=eps_col[:])
            rstd = statp.tile([128, 1], FP32, tag="rstd")
            nc.scalar.activation(rstd[:], lnv[:],
                                 mybir.ActivationFunctionType.Exp, scale=-0.5)
            nmr = statp.tile([128, 1], FP32, tag="nmr")
            nc.vector.scalar_tensor_tensor(nmr[:], agg[:, 0:1], -1.0,
                                           rstd[:], mybir.AluOpType.mult,
                                           mybir.AluOpType.mult)
            xn = xio.tile([128, D], BF16, tag="xn")
            nc.scalar.activation(xn[:], xt[:],
                                 mybir.ActivationFunctionType.Identity,
                                 bias=nmr[:], scale=rstd[:])
            for g in range(2):  # groups of 4 d-chunks
                tpt = tp.tile([128, 512], BF16, tag="tpb")
                for i in range(4):
                    dc = g * 4 + i
                    nc.tensor.transpose(tpt[:, i * 128:(i + 1) * 128],
                                        xn[:, dc * 128:(dc + 1) * 128],
                                        identb[:])
                dst = xnt[:].rearrange("p (dc tt) -> p dc tt", tt=TL)[
                    :, g * 4:(g + 1) * 4, t * 128:(t + 1) * 128]
                dst8 = xnt8[:].rearrange("p (dc tt) -> p dc tt", tt=TL)[
                    :, g * 4:(g + 1) * 4, t * 128:(t + 1) * 128]
                src_ = tpt[:].rearrange("p (i c) -> p i c", c=128)
                nc.scalar.copy(dst, src_)
                nc.vector.tensor_copy(dst8, src_)
        es_rows.close()

        if TRUNC == 2:
            finish_tail([xnt[0:1, 0:8].bitcast(FP32)])
            es_xnt.close()
            return

        # ---------------- stage 1: k/v proj + exp + ctx ----------------
        es_wkv = ExitStack()
        wkvp = es_wkv.enter_context(tc.tile_pool(name="wkv", bufs=1))
        kvp = es_wkv.enter_context(tc.tile_pool(name="kv", bufs=2))
        wk8 = wkvp.tile([128, ND * D], FP8)
        nc.sync.dma_start(
            out=wk8[:].rearrange("p (dc c) -> p dc c", c=D),
            in_=wk_d[:].rearrange("(dc p) c -> p dc c", p=128))
        wv = wkvp.tile([128, ND * D], BF16)
        nc.sync.dma_start(
            out=wv[:].rearrange("p (dc c) -> p dc c", c=D),
            in_=wv_d[:].rearrange("(dc p) c -> p dc c", p=128))

        ctx_sb = constp.tile([128, 8 * 65], FP32)
        xnt8v = xnt8[:].rearrange("p (dc tt) -> p dc tt", tt=TL)
        wk8v = wk8[:].rearrange("p (dc c) -> p dc c", c=D)

        def em_kv(t):
            ke = kvp.tile([128, D], BF16, tag="ke")
            va = kvp.tile([128, H * 66], BF16, tag="va")
            for jh in range(2):
                kh = pp.tile([128, 512], FP32, tag="pp")
                for i in range(4):
                    nc.tensor.matmul(
                        kh[:],
                        xnt8v[:, 2 * i:2 * i + 2, t * 128:(t + 1) * 128],
                        wk8v[:, 2 * i:2 * i + 2, jh * 512:(jh + 1) * 512],
                        start=(i == 0), stop=(i == 3 and not has_ck),
                        perf_mode=DR)
                if has_ck:
                    nc.tensor.matmul(kh[:], ones_row[:, 0:128],
                                     ck_row[:, jh * 512:(jh + 1) * 512],
                                     start=False, stop=True)
                nc.scalar.activation(ke[:, jh * 512:(jh + 1) * 512], kh[:],
                                     mybir.ActivationFunctionType.Exp,
                                     scale=1.0 / WS)
            lhss = [xnt[:, dc * TL + t * 128: dc * TL + (t + 1) * 128]
                    for dc in range(ND)]
            for jh in range(2):
                vh = pp.tile([128, 512], FP32, tag="pp")
                for dc in range(ND):
                    nc.tensor.matmul(
                        vh[:], lhss[dc],
                        wv[:, dc * D + jh * 512:dc * D + (jh + 1) * 512],
                        start=(dc == 0), stop=(dc == 7 and not has_cv))
                if has_cv:
                    nc.tensor.matmul(vh[:], ones_row[:, 0:128],
                                     cv_row[:, jh * 512:(jh + 1) * 512],
                                     start=False, stop=True)
                nc.scalar.copy(
                    va[:].rearrange("p (h l) -> p h l", l=66)[
                        :, jh * 8:(jh + 1) * 8, 0:64],
                    vh[:].rearrange("p (h l) -> p h l", l=64))
            nc.vector.memset(
                va[:].rearrange("p (h l) -> p h l", l=66)[:, :, 64:65], 1.0)
            # one single-shot matmul per psum tile (bank sharing between
            # single-shot groups corrupts data on HW), then DVE-accumulate
            # into the SBUF ctx accumulator. Heads pack pairwise by parity.
            for hp in range(H // 2):
                cp = pp.tile([128, 512], FP32, tag="pp")
                for par in range(2):
                    h = 2 * hp + par
                    nc.tensor.matmul(cp[par * 64:par * 64 + 64, 0:65],
                                     ke[:, h * 64:(h + 1) * 64],
                                     va[:, h * 66:h * 66 + 65])
                if t == 0:
                    nc.vector.tensor_copy(
                        ctx_sb[:, hp * 65:(hp + 1) * 65], cp[:, 0:65])
                else:
                    nc.vector.tensor_tensor(
                        ctx_sb[:, hp * 65:(hp + 1) * 65],
                        ctx_sb[:, hp * 65:(hp + 1) * 65],
                        cp[:, 0:65], mybir.AluOpType.add)

        for tt in range(NT + 2):
            if tt < NT:
                em_s0(tt)
            if tt >= 2:
                em_kv(tt - 2)

        # ---------------- stage 3a: stage ctx + pairwise AllReduce ----------------
        cc_in = dramp.tile([H, 64, 65], FP32)
        cc_out = dramp.tile([H, 64, 65], FP32)
        for q in range(2):
            nc.sync.dma_start(
                out=cc_in[:].rearrange("(g q) d l -> q d g l", q=2)[q],
                in_=ctx_sb[q * 64:(q + 1) * 64, :].rearrange(
                    "d (g l) -> d g l", l=65))
        if USE_COLLECTIVE:
            nc.gpsimd.collective_compute(
                "AllReduce", mybir.AluOpType.add,
                replica_groups=[[0, 1], [2, 3], [4, 5], [6, 7]],
                ins=[cc_in.opt()], outs=[cc_out.opt()])
        else:
            nc.sync.dma_start(out=cc_out[:], in_=cc_in[:])
        es_wkv.close()

        if TRUNC == 3:
            ctxn3 = constp.tile([128, 8 * 65], FP32)
            for q in range(2):
                nc.sync.dma_start(
                    out=ctxn3[q * 64:(q + 1) * 64, :].rearrange(
                        "d (g l) -> d g l", l=65),
                    in_=cc_out[:].rearrange("(g q) d l -> q d g l", q=2)[q])
            finish_tail([ctxn3[0:1, 0:4], xnt[0:1, 0:4].bitcast(FP32)])
            es_xnt.close()
            return

        # ---------------- stage 2: q proj + exp (overlaps collective) ----------------
        es_wq = ExitStack()
        es_qt = ExitStack()
        wqp = es_wq.enter_context(tc.tile_pool(name="wq", bufs=1))
        qtp = es_qt.enter_context(tc.tile_pool(name="qT", bufs=1, side="right"))
        wq8 = wqp.tile([128, ND * D], FP8)
        nc.sync.dma_start(
            out=wq8[:].rearrange("p (dc c) -> p dc c", c=D),
            in_=wq_d[:].rearrange("(dc p) c -> p dc c", p=128))
        wq8v = wq8[:].rearrange("p (dc c) -> p dc c", c=D)
        qT = qtp.tile([128, ND * TL], BF16)  # j-chunk jc at cols jc*TL
        for jc in range(ND):
            for th in range(TL // 512):
                qps = pp.tile([128, 512], FP32, tag="pp")
                for i in range(4):
                    nc.tensor.matmul(
                        qps[:],
                        wq8v[:, 2 * i:2 * i + 2, jc * 128:jc * 128 + 128],
                        xnt8v[:, 2 * i:2 * i + 2, th * 512:(th + 1) * 512],
                        start=(i == 0), stop=(i == 3 and not has_cq),
                        perf_mode=DR)
                if has_cq:
                    nc.tensor.matmul(qps[:],
                                     cq_row[:, jc * 128:(jc + 1) * 128],
                                     ones_row[:], start=False, stop=True)
                nc.scalar.activation(
                    qT[:, jc * TL + th * 512:jc * TL + (th + 1) * 512],
                    qps[:], mybir.ActivationFunctionType.Exp, scale=1.0 / WS)
        es_wq.close()
        es_xnt.close()

        if TRUNC == 4:
            finish_tail([qT[0:1, 0:8].bitcast(FP32)])
            es_qt.close()
            return

        # ---------------- stage 4: readback + normalize ctx ----------------
        ctxn = constp.tile([128, 8 * 65], FP32)
        for q in range(2):
            nc.sync.dma_start(
                out=ctxn[q * 64:(q + 1) * 64, :].rearrange(
                    "d (g l) -> d g l", l=65),
                in_=cc_out[:].rearrange("(g q) d l -> q d g l", q=2)[q])
        rk = statp.tile([128, 8], FP32, tag="rk")
        nc.vector.reciprocal(
            rk[:], ctxn[:].rearrange("p (g l) -> p g l", l=65)[:, :, 64])
        ctx_aug = constp.tile([128, 8 * 66], BF16)
        for g in range(8):
            nc.vector.tensor_scalar(ctx_aug[:, g * 66:g * 66 + 64],
                                    ctxn[:, g * 65:g * 65 + 64],
                                    rk[:, g:g + 1], None,
                                    mybir.AluOpType.mult)
        nc.vector.memset(
            ctx_aug[:].rearrange("p (g l) -> p g l", l=66)[:, :, 64:65], 1.0)

        if TRUNC == 5:
            finish_tail([ctx_aug[0:1, 0:8].bitcast(FP32), qT[0:1, 0:8].bitcast(FP32)])
            es_qt.close()
            return

        # ---------------- stage 5+6 fused per token tile ----------------
        es_out = ExitStack()
        wop = es_out.enter_context(tc.tile_pool(name="wo", bufs=1, side="right"))
        hyp = es_out.enter_context(tc.tile_pool(name="hy", bufs=3, side="right"))
        wo = wop.tile([128, ND * D], BF16)
        nc.sync.dma_start(
            out=wo[:].rearrange("p (dc c) -> p dc c", c=D),
            in_=wo_d[:].rearrange("(dc p) c -> p dc c", p=128))

        yts, hss = {}, {}

        def em_y(t):
            yt = hyp.tile([128, D], FP32, tag="yt")
            rq = statp.tile([128, 16], FP32, tag="rq")
            yts[t] = yt
            for h in range(H):
                par = h % 2
                yp = pp.tile([128, 512], FP32, tag="pp")
                nc.tensor.matmul(
                    yp[:, 0:65],
                    qT[par * 64:par * 64 + 64,
                       (h // 2) * TL + t * 128:(h // 2) * TL + (t + 1) * 128],
                    ctx_aug[par * 64:par * 64 + 64,
                            (h // 2) * 66:(h // 2) * 66 + 65])
                nc.vector.reciprocal(rq[:, h:h + 1], yp[:, 64:65])
                if h % 2 == 0:
                    nc.scalar.mul(yt[:, h * 64:(h + 1) * 64], yp[:, 0:64],
                                  rq[:, h:h + 1])
                else:
                    nc.vector.tensor_scalar(yt[:, h * 64:(h + 1) * 64],
                                            yp[:, 0:64], rq[:, h:h + 1],
                                            None, mybir.AluOpType.mult)

        def em_ln(t):
            yt = yts.pop(t)
            st6 = statp.tile([128, 2, 6], FP32, tag="st6")
            nc.vector.bn_stats(st6[:, 0, :], yt[:, 0:512])
            nc.vector.bn_stats(st6[:, 1, :], yt[:, 512:1024])
            agg = statp.tile([128, 2], FP32, tag="agg")
            nc.vector.bn_aggr(agg[:], st6[:])
            # rstd = rsqrt(var+eps) via quake seed + 2 Newton steps, all DVE
            # (keeps Sqrt's act table off the Scalar engine in this phase).
            q0 = statp.tile([128, 8], FP32, tag="qk")
            v0, s1 = q0[:, 0:1], q0[:, 1:2]
            z = q0[:, 2:3]
            nc.vector.tensor_scalar(v0, agg[:, 1:2], EPS, None,
                                    mybir.AluOpType.add)
            nc.vector.tensor_scalar(s1.bitcast(I32), v0.bitcast(I32), 1, None,
                                    mybir.AluOpType.logical_shift_right)
            nc.vector.tensor_tensor(z.bitcast(I32), kq_i[:], s1.bitcast(I32),
                                    mybir.AluOpType.subtract)
            for it in range(2):
                b_ = q0[:, 3 + 2 * it:4 + 2 * it]
                c_ = q0[:, 4 + 2 * it:5 + 2 * it]
                nc.vector.tensor_tensor(b_, z, z, mybir.AluOpType.mult)
                nc.vector.tensor_tensor(c_, v0, b_, mybir.AluOpType.mult)
                nc.vector.tensor_scalar(c_, c_, -0.5, 1.5,
                                        mybir.AluOpType.mult,
                                        mybir.AluOpType.add)
                zn = q0[:, 7:8] if it == 1 else q0[:, 2:3]
                nc.vector.tensor_tensor(zn, z, c_, mybir.AluOpType.mult)
                z = zn
            nmr = statp.tile([128, 1], FP32, tag="nmr")
            nc.vector.scalar_tensor_tensor(nmr[:], agg[:, 0:1], -1.0,
                                           z, mybir.AluOpType.mult,
                                           mybir.AluOpType.mult)
            ln = hyp.tile([128, D], BF16, tag="ln")
            nc.scalar.activation(ln[:], yt[:],
                                 mybir.ActivationFunctionType.Identity,
                                 bias=nmr[:], scale=z)
            nc.vector.tensor_tensor(ln[:], ln[:], s2_b[:],
                                    mybir.AluOpType.mult)
            nc.vector.tensor_tensor(ln[:], ln[:], sh2_b[:],
                                    mybir.AluOpType.add)
            hs = hyp.tile([128, D], BF16, tag="hs")
            if USE_NATIVE_SILU:
                nc.scalar.activation(hs[:], ln[:],
                                     mybir.ActivationFunctionType.Silu)
            else:
                nc.scalar.activation(hs[:], ln[:],
                                     mybir.ActivationFunctionType.Sigmoid)
                nc.vector.tensor_tensor(hs[:], hs[:], ln[:],
                                        mybir.AluOpType.mult)
            hss[t] = hs

        def em_out(t):
            hs = hss.pop(t)
            hst = hyp.tile([128, D], BF16, tag="hst")
            for g in range(2):
                tpt = tp.tile([128, 512], BF16, tag="tpb")
                for i in range(4):
                    dc = g * 4 + i
                    nc.tensor.transpose(tpt[:, i * 128:(i + 1) * 128],
                                        hs[:, dc * 128:(dc + 1) * 128],
                                        identb[:])
                nc.scalar.copy(hst[:, g * 512:(g + 1) * 512], tpt[:])
            xt2 = xio.tile([128, D], FP32, tag="xin")
            nc.gpsimd.dma_start(out=xt2[:], in_=x_d[t * 128:(t + 1) * 128, :])
            fin = xio.tile([128, D], FP32, tag="fin")
            for jh in range(2):
                oph = pp.tile([128, 512], FP32, tag="pp")
                for dc in range(ND):
                    nc.tensor.matmul(
                        oph[:], hst[:, dc * 128:(dc + 1) * 128],
                        wo[:, dc * D + jh * 512:dc * D + (jh + 1) * 512],
                        start=(dc == 0), stop=(dc == 7 and not has_co))
                if has_co:
                    nc.tensor.matmul(oph[:], ones_row[:, 0:128],
                                     co_row[:, jh * 512:(jh + 1) * 512],
                                     start=False, stop=True)
                nc.vector.tensor_tensor(fin[:, jh * 512:(jh + 1) * 512],
                                        oph[:], xt2[:, jh * 512:(jh + 1) * 512],
                                        mybir.AluOpType.add)
            nc.gpsimd.dma_start(out=out_d[t * 128:(t + 1) * 128, :], in_=fin[:])

        for tt in range(NT + 2):
            if tt < NT:
                em_y(tt)
            if 1 <= tt <= NT:
                em_ln(tt - 1)
            if tt >= 2:
                em_out(tt - 2)
        es_out.close()
        es_qt.close()

    with tile.TileContext(nc) as tc, ExitStack() as es:
        _emit(tc, es)
    nc.compile()
    _legalize_waits(nc)
    return nc


def kernel(**inputs):
    x = np.asarray(inputs["x"], np.float32)
    emb = np.asarray(inputs["emb"], np.float32)
    gate_msa = np.asarray(inputs["gate_msa"], np.float32)
    norm_g = np.asarray(inputs["norm_g"], np.float32)
    norm_b = np.asarray(inputs["norm_b"], np.float32)
    Wq = np.asarray(inputs["Wq"], np.float32)
    bq = np.asarray(inputs["bq"], np.float32)
    Wk = np.asarray(inputs["Wk"], np.float32)
    bk = np.asarray(inputs["bk"], np.float32)
    Wv = np.asarray(inputs["Wv"], np.float32)
    bv = np.asarray(inputs["bv"], np.float32)
    emb_W = np.asarray(inputs["emb_W"], np.float32)
    emb_b = np.asarray(inputs["emb_b"], np.float32)
    sn_g = np.asarray(inputs["sn_g"], np.float32)
    sn_b = np.asarray(inputs["sn_b"], np.float32)
    out_W = np.asarray(inputs["out_W"], np.float32)
    out_b = np.asarray(inputs["out_b"], np.float32)

    import ml_dtypes

    # fold layernorm affine into projection weights; fp8 k/q pre-scaled by WS
    def to8(w):
        return np.ascontiguousarray(
            np.clip(w * WS, -240.0, 240.0).astype(ml_dtypes.float8_e4m3))

    wk8 = to8(norm_g[:, None] * Wk)
    wq8 = to8(norm_g[:, None] * Wq)
    wv_f = np.ascontiguousarray(
        (norm_g[:, None] * Wv).astype(ml_dtypes.bfloat16))
    embw_bf = emb_W.astype(ml_dtypes.bfloat16)
    cq = (norm_b @ Wq + bq) * WS
    ck = (norm_b @ Wk + bk) * WS
    cv = norm_b @ Wv + bv

    flags = (bool(np.any(cq)), bool(np.any(ck)), bool(np.any(cv)),
             bool(np.any(out_b)), bool(np.any(emb_b)))
    if flags not in _CACHE:
        _CACHE[flags] = build(*flags)
    nc = _CACHE[flags]

    in_maps = []
    for c in range(NCORES):
        b, half = c // 2, c % 2
        # fold the per-batch msa gate into the output projection columns
        wo_fold = np.ascontiguousarray(
            (out_W * gate_msa[b, 0, :][None, :]).astype(ml_dtypes.bfloat16))
        m = {
            "x": np.ascontiguousarray(x[b, half * TL:(half + 1) * TL, :]),
            "wq8": wq8, "wk8": wk8, "wv": wv_f, "wo": wo_fold,
            "embw": embw_bf,
            "embt": np.ascontiguousarray(emb[b, 0, :].reshape(ND, 128).T.ravel()),
            "gsn": np.ascontiguousarray(np.stack([sn_g, sn_b])),
        }
        if flags[0]:
            m["cq"] = cq
        if flags[1]:
            m["ck"] = ck
        if flags[2]:
            m["cv"] = cv
        if flags[3]:
            m["co"] = out_b * gate_msa[b, 0, :]
        if flags[4]:
            m["cemb"] = emb_b
        in_maps.append(m)

    res = run_bass_kernel_spmd(nc, in_maps, core_ids=list(range(NCORES)),
                               **_RUN_KW)
    kernel.last_result = res
    out = np.stack([res.results[c]["out"] for c in range(NCORES)])
    return out.reshape(B, 2, TL, D).reshape(B, T, D)


_RUN_KW = {}
kernel.last_result = None


# revision 15
# speedup vs baseline: 1.2110x; 1.2110x over previous
"""DitLinearTemporalSelfAttention on 8 TRN2 NeuronCores (Bass/Tile).

Sharding: token-parallel. Core c handles batch b=c//2, token half c%2
(2048 tokens, full D=1024). The temporal-softmax/context reduction over
T=4096 spans two cores per batch -> pairwise AllReduce [[0,1],[2,3],...]
of the tiny per-batch [H,dh,dh+1] context+ksum buffer (266 KB).

Per core (tokens t in its slice):
  xn    = LN(x) bf16 (rsqrt via quake+NR on DVE), norm_g folded host-side
  kT    : fp8 DoubleRow projection (xnT8 x Wk8*64), exp(psum/64) -> ke bf16
  vv    : bf16 projection -> va pair-packed [128, 8*130] with ones cols
  ctx   : head-pair block matmuls accumulated IN PSUM across all 16 token
          tiles (3 banks, single start per bank), ksum via ones columns
  (pairwise AllReduce of [H,64,65])
  qT    : fp8 DoubleRow projection out[j,t], exp(psum/64) -> qT bf16
  y     : block-diag ctx_aug2 [128, 8*130]; per-tile 3-bank psum chains;
          bulk 1/qden via strided reciprocal + broadcast multiply
  h     = silu(LN(y)*scale2 + shift2)
  out   = hs @ (gate*out_W) + x, residual added in PSUM via identity
          matmul of xb (bf16 copy of x), output DMA'd straight from PSUM
"""

import numpy as np

import concourse.bass as bass
import concourse.bacc as bacc
import concourse.mybir as mybir
import concourse.tile as tile
from concourse import masks
from concourse.bass_utils import run_bass_kernel_spmd

B, T, D, H, DH = 4, 4096, 1024, 16, 64
NCORES = 8
TL = T // 2          # tokens per core
NT = TL // 128       # 16 token tiles
ND = D // 128        # 8 d-chunks
NP = H // 2          # 8 head pairs
EPS = 1e-5
WS = 64.0            # fp8 weight pre-scale (pow2; unscaled via exp scale)
FP32 = mybir.dt.float32
FP32R = mybir.dt.float32r
BF16 = mybir.dt.bfloat16
FP8 = mybir.dt.float8e4
I32 = mybir.dt.int32
DR = mybir.MatmulPerfMode.DoubleRow
QUAKE = 0x5F3759DF
# ctx/y psum bank composition: pairs per bank
BANKS = [(0, 3), (3, 3), (6, 2)]

_CACHE = {}
USE_COLLECTIVE = True
USE_NATIVE_SILU = True

def r32(ap):
    return ap.bitcast(FP32R)


def _legalize_waits(nc, cap=2, escap=2):
    """Split >cap semaphore waits off any instruction into EventSemaphore
    instructions placed immediately before it on the same engine (walrus
    codegen structs hold only a few sync-wait slots)."""
    n = 0
    for bb in nc.main_func.blocks:
        out = []
        changed = False
        for ins in bb.instructions:
            si = ins.sync_info
            ty = type(ins).__name__
            icap = 1 if ty == "InstDMACopy" else cap
            if (si is not None and si.on_wait is not None
                    and len(si.on_wait) > icap
                    and ty not in ("InstDrain", "InstEventSemaphore")):
                waits = list(si.on_wait)
                keep, extra = waits[:icap], waits[icap:]
                while extra:
                    chunk, extra = extra[:escap], extra[escap:]
                    n += 1
                    es = mybir.InstEventSemaphore(
                        name=f"I-wsplit-{n}", engine=ins.engine,
                        sync_info=mybir.SyncInfo(on_wait=list(chunk),
                                                 on_update=[]))
                    out.append(es)
                ins.sync_info = mybir.SyncInfo(
                    on_wait=keep, on_update=list(si.on_update or []))
                changed = True
            out.append(ins)
        if changed:
            bb.instructions = out
    return n


def build(has_cq, has_ck, has_cv, has_co, has_cemb):
    from contextlib import ExitStack

    nc = bacc.Bacc("TRN2", target_bir_lowering=False, debug=False,
                   num_devices=NCORES)

    x_d = nc.dram_tensor("x", [TL, D], FP32, kind="ExternalInput")
    xb_d = nc.dram_tensor("xb", [TL, D], BF16, kind="ExternalInput")
    wk_d = nc.dram_tensor("wk8", [D, D], FP8, kind="ExternalInput")
    wv_d = nc.dram_tensor("wv", [D, D], BF16, kind="ExternalInput")
    wq_d = nc.dram_tensor("wq8", [D, D], FP8, kind="ExternalInput")
    wo_d = nc.dram_tensor("wo", [D, D], BF16, kind="ExternalInput")
    embw_d = nc.dram_tensor("embw", [D, 2 * D], BF16, kind="ExternalInput")
    embt_d = nc.dram_tensor("embt", [D], FP32, kind="ExternalInput")
    gsn_d = nc.dram_tensor("gsn", [2, D], FP32R, kind="ExternalInput")
    cemb_d = nc.dram_tensor("cemb", [2 * D], FP32R, kind="ExternalInput") if has_cemb else None
    cq_d = nc.dram_tensor("cq", [D], FP32R, kind="ExternalInput") if has_cq else None
    ck_d = nc.dram_tensor("ck", [D], FP32R, kind="ExternalInput") if has_ck else None
    cv_d = nc.dram_tensor("cv", [D], FP32R, kind="ExternalInput") if has_cv else None
    co_d = nc.dram_tensor("co", [D], FP32R, kind="ExternalInput") if has_co else None
    out_d = nc.dram_tensor("out", [TL, D], FP32, kind="ExternalOutput")

    def _emit(tc, es):
        constp = es.enter_context(tc.tile_pool(name="const", bufs=1))
        xio = es.enter_context(tc.tile_pool(name="xio", bufs=2))
        statp = es.enter_context(tc.tile_pool(name="stat", bufs=4))
        dramp = es.enter_context(tc.tile_pool(name="dram", bufs=1, space="DRAM"))
        tp = es.enter_context(tc.tile_pool(name="tp", bufs=2, space="PSUM"))
        pp = es.enter_context(tc.tile_pool(name="pp", bufs=3, space="PSUM"))

        # ---------------- constants ----------------
        ident = constp.tile([128, 128], FP32)
        masks.make_identity(nc, ident[:])
        identb = constp.tile([128, 128], BF16)
        nc.vector.tensor_copy(identb[:], ident[:])
        ones_row32 = constp.tile([1, 512], FP32)
        nc.vector.memset(ones_row32[:], 1.0)
        ones_row = constp.tile([1, 512], FP32R)
        nc.vector.tensor_copy(ones_row[:], ones_row32[:])
        kq_i = constp.tile([128, 1], I32)
        nc.vector.memset(kq_i[:], QUAKE)

        # rows: sng/snb via ONE dma; bias rows when present
        gsn = constp.tile([1, 2 * D], FP32R)
        nc.sync.dma_start(out=gsn[:], in_=gsn_d[:].rearrange("a b -> (a b)").unsqueeze(0))
        sng_row = gsn[:, 0:D]
        snb_row = gsn[:, D:2 * D]

        def load_row(pool, dram_ap, n):
            t_ = pool.tile([1, n], FP32R, tag=dram_ap.tensor.name)
            nc.sync.dma_start(out=t_[:], in_=dram_ap.unsqueeze(0))
            return t_

        cemb_row = load_row(constp, cemb_d.ap(), 2 * D) if has_cemb else None
        cq_row = load_row(constp, cq_d.ap(), D) if has_cq else None
        ck_row = load_row(constp, ck_d.ap(), D) if has_ck else None
        cv_row = load_row(constp, cv_d.ap(), D) if has_cv else None
        co_row = load_row(constp, co_d.ap(), D) if has_co else None

        # quake rsqrt: z = rsqrt(v0) via bit seed + 2 Newton steps, all DVE
        def rsqrt_dve(scr, var_ap):
            v0, s1, z = scr[:, 0:1], scr[:, 1:2], scr[:, 2:3]
            nc.vector.tensor_scalar(v0, var_ap, EPS, None,
                                    mybir.AluOpType.add)
            nc.vector.tensor_scalar(s1.bitcast(I32), v0.bitcast(I32), 1, None,
                                    mybir.AluOpType.logical_shift_right)
            nc.vector.tensor_tensor(z.bitcast(I32), kq_i[:], s1.bitcast(I32),
                                    mybir.AluOpType.subtract)
            for it in range(2):
                b_ = scr[:, 3 + 2 * it:4 + 2 * it]
                c_ = scr[:, 4 + 2 * it:5 + 2 * it]
                nc.vector.tensor_tensor(b_, z, z, mybir.AluOpType.mult)
                nc.vector.tensor_tensor(c_, v0, b_, mybir.AluOpType.mult)
                nc.vector.tensor_scalar(c_, c_, -0.5, 1.5,
                                        mybir.AluOpType.mult,
                                        mybir.AluOpType.add)
                zn = scr[:, 7:8] if it == 1 else scr[:, 2:3]
                nc.vector.tensor_tensor(zn, z, c_, mybir.AluOpType.mult)
                z = zn
            return z

        # xnT opens BEFORE setup transients so it never reuses their zone
        es_xnt = ExitStack()
        xntp = es_xnt.enter_context(tc.tile_pool(name="xnT", bufs=1))
        xnt = xntp.tile([128, ND * TL], BF16)
        xnt8 = xntp.tile([128, ND * TL], FP8)

        es_rows = ExitStack()
        rowsp = es_rows.enter_context(tc.tile_pool(name="rows", bufs=1))

        # embt host-permuted: one DMA fills [128, 8], (p, c) = emb[c*128+p]
        embt_sb = rowsp.tile([128, ND], FP32)
        nc.sync.dma_start(out=embt_sb[:], in_=embt_d[:].rearrange(
            "(p c) -> p c", c=ND))
        silu_e = rowsp.tile([128, ND], FP32)
        if USE_NATIVE_SILU:
            nc.scalar.activation(silu_e[:], embt_sb[:],
                                 mybir.ActivationFunctionType.Silu)
        else:
            nc.scalar.activation(silu_e[:], embt_sb[:],
                                 mybir.ActivationFunctionType.Sigmoid)
            nc.vector.tensor_tensor(silu_e[:], silu_e[:], embt_sb[:],
                                    mybir.AluOpType.mult)

        # ------------- emb MLP: bf16 matvec on PE (one embw DMA) -------------
        silu_eb = rowsp.tile([128, 2 * ND], BF16)
        nc.vector.tensor_copy(
            silu_eb[:].rearrange("p (c two) -> p c two", two=2)[:, :, 0:1],
            silu_e[:].unsqueeze(2))
        embw = rowsp.tile([128, ND * 2 * D], BF16)  # d-chunk dc at cols dc*2048
        nc.sync.dma_start(
            out=embw[:].rearrange("p (dc c) -> p dc c", c=2 * D),
            in_=embw_d[:].rearrange("(dc p) c -> p dc c", p=128))
        emb_sel = rowsp.tile([1, 2 * D], FP32R)
        for nch in range(4):
            epn = pp.tile([1, 512], FP32, tag="pp")
            for dc in range(ND):
                nc.tensor.matmul(epn[:],
                                 silu_eb[:, 2 * dc:2 * dc + 1],
                                 embw[:, dc * 2048 + nch * 512:dc * 2048 + (nch + 1) * 512],
                                 start=(dc == 0), stop=(dc == ND - 1))
            nc.vector.tensor_copy(emb_sel[:, nch * 512:(nch + 1) * 512], epn[:])
        if has_cemb:
            nc.vector.tensor_tensor(emb_sel[:], emb_sel[:], cemb_row[:],
                                    mybir.AluOpType.add)
        # broadcast emb_sel + sng/snb rows to all partitions
        emb_sel_b = rowsp.tile([128, 2 * D], FP32)
        for nch in range(4):
            bp = pp.tile([128, 512], FP32, tag="pp")
            nc.tensor.matmul(bp[:], ones_row[:, 0:128],
                             emb_sel[:, nch * 512:(nch + 1) * 512])
            nc.vector.tensor_copy(emb_sel_b[:, nch * 512:(nch + 1) * 512], bp[:])

        def bcast(row, name):
            out = constp.tile([128, D], FP32, tag=f"bc_{name}")
            for nh in range(2):
                bp = pp.tile([128, 512], FP32, tag="pp")
                nc.tensor.matmul(bp[:], ones_row[:, 0:128],
                                 row[:, nh * 512:(nh + 1) * 512])
                nc.vector.tensor_copy(out[:, nh * 512:(nh + 1) * 512], bp[:])
            return out

        sng_b = bcast(sng_row, "sng")
        snb_b = bcast(snb_row, "snb")
        # scale2 = sng*(1+scale); shift2 = snb*(1+scale) + shift
        t1_b = rowsp.tile([128, D], FP32)
        nc.vector.tensor_scalar(t1_b[:], emb_sel_b[:, 0:D], 1.0, None,
                                mybir.AluOpType.add)
        s2_b = constp.tile([128, D], FP32)
        nc.vector.tensor_tensor(s2_b[:], t1_b[:], sng_b[:],
                                mybir.AluOpType.mult)
        sh2_b = constp.tile([128, D], FP32)
        nc.vector.tensor_tensor(sh2_b[:], t1_b[:], snb_b[:],
                                mybir.AluOpType.mult)
        nc.vector.tensor_tensor(sh2_b[:], sh2_b[:], emb_sel_b[:, D:2 * D],
                                mybir.AluOpType.add)

        # ---------------- stage 0 emitter: load x, LN, transpose ----------------
        def em_s0(t):
            xt = xio.tile([128, D], FP32, tag="xin")
            nc.sync.dma_start(out=xt[:], in_=x_d[t * 128:(t + 1) * 128, :])
            st6 = statp.tile([128, 2, 6], FP32, tag="st6")
            nc.vector.bn_stats(st6[:, 0, :], xt[:, 0:512])
            nc.vector.bn_stats(st6[:, 1, :], xt[:, 512:1024])
            agg = statp.tile([128, 2], FP32, tag="agg")
            nc.vector.bn_aggr(agg[:], st6[:])
            scr = statp.tile([128, 8], FP32, tag="qk")
            rstd = rsqrt_dve(scr, agg[:, 1:2])
            nmr = statp.tile([128, 1], FP32, tag="nmr")
            nc.vector.scalar_tensor_tensor(nmr[:], agg[:, 0:1], -1.0,
                                           rstd, mybir.AluOpType.mult,
                                           mybir.AluOpType.mult)
            xn = xio.tile([128, D], BF16, tag="xn")
            nc.scalar.activation(xn[:], xt[:],
                                 mybir.ActivationFunctionType.Identity,
                                 bias=nmr[:], scale=rstd)
            for g in range(2):  # groups of 4 d-chunks
                tpt = tp.tile([128, 512], BF16, tag="tpb")
                for i in range(4):
                    dc = g * 4 + i
                    nc.tensor.transpose(tpt[:, i * 128:(i + 1) * 128],
                                        xn[:, dc * 128:(dc + 1) * 128],
                                        identb[:])
                dst = xnt[:].rearrange("p (dc tt) -> p dc tt", tt=TL)[
                    :, g * 4:(g + 1) * 4, t * 128:(t + 1) * 128]
                dst8 = xnt8[:].rearrange("p (dc tt) -> p dc tt", tt=TL)[
                    :, g * 4:(g + 1) * 4, t * 128:(t + 1) * 128]
                src_ = tpt[:].rearrange("p (i c) -> p i c", c=128)
                nc.scalar.copy(dst, src_)
                nc.vector.tensor_copy(dst8, src_)
        es_rows.close()

        # ---------------- stage 1: k/v proj + exp + ctx (psum-chained) --------
        es_wkv = ExitStack()
        wkvp = es_wkv.enter_context(tc.tile_pool(name="wkv", bufs=1))
        kvp = es_wkv.enter_context(tc.tile_pool(name="kv", bufs=3))
        ctxp = es_wkv.enter_context(tc.tile_pool(name="ctx", bufs=1, space="PSUM"))
        wk8 = wkvp.tile([128, ND * D], FP8)
        nc.sync.dma_start(
            out=wk8[:].rearrange("p (dc c) -> p dc c", c=D),
            in_=wk_d[:].rearrange("(dc p) c -> p dc c", p=128))
        wv = wkvp.tile([128, ND * D], BF16)
        nc.sync.dma_start(
            out=wv[:].rearrange("p (dc c) -> p dc c", c=D),
            in_=wv_d[:].rearrange("(dc p) c -> p dc c", p=128))

        ctxb = []
        for b, (p0, n) in enumerate(BANKS):
            cbt = ctxp.tile([128, n * 130], FP32, tag=f"ctx{b}", name=f"ctx{b}")
            ctxb.append(cbt)
        xnt8v = xnt8[:].rearrange("p (dc tt) -> p dc tt", tt=TL)
        wk8v = wk8[:].rearrange("p (dc c) -> p dc c", c=D)

        kes, vas = {}, {}

        def em_kv(t):
            ke = kvp.tile([128, D], BF16, tag="ke")
            va = kvp.tile([128, NP * 130], BF16, tag="va")
            kes[t], vas[t] = ke, va
            for jh in range(2):
                kh = pp.tile([128, 512], FP32, tag="pp")
                for i in range(4):
                    nc.tensor.matmul(
                        kh[:],
                        xnt8v[:, 2 * i:2 * i + 2, t * 128:(t + 1) * 128],
                        wk8v[:, 2 * i:2 * i + 2, jh * 512:(jh + 1) * 512],
                        start=(i == 0), stop=(i == 3 and not has_ck),
                        perf_mode=DR)
                if has_ck:
                    nc.tensor.matmul(kh[:], ones_row[:, 0:128],
                                     ck_row[:, jh * 512:(jh + 1) * 512],
                                     start=False, stop=True)
                nc.scalar.activation(ke[:, jh * 512:(jh + 1) * 512], kh[:],
                                     mybir.ActivationFunctionType.Exp,
                                     scale=1.0 / WS)
            lhss = [xnt[:, dc * TL + t * 128: dc * TL + (t + 1) * 128]
                    for dc in range(ND)]
            for jh in range(2):
                vh = pp.tile([128, 512], FP32, tag="pp")
                for dc in range(ND):
                    nc.tensor.matmul(
                        vh[:], lhss[dc],
                        wv[:, dc * D + jh * 512:dc * D + (jh + 1) * 512],
                        start=(dc == 0), stop=(dc == 7 and not has_cv))
                if has_cv:
                    nc.tensor.matmul(vh[:], ones_row[:, 0:128],
                                     cv_row[:, jh * 512:(jh + 1) * 512],
                                     start=False, stop=True)
                # pairs 4jh..4jh+3: v values into [g, q, 0:64] of va
                nc.scalar.copy(
                    va[:, 4 * jh * 130:(4 * jh + 4) * 130].rearrange(
                        "p (g q l) -> p g q l", q=2, l=65)[:, :, :, 0:64],
                    vh[:].rearrange("p (g l) -> p g l", l=64).rearrange(
                        "p (g q) l -> p g q l", q=2))
            nc.vector.memset(
                va[:].rearrange("p (g q l) -> p g q l", q=2, l=65)[:, :, :, 64:65],
                1.0)

        def em_ctx(t):
            ke, va = kes.pop(t), vas.pop(t)
            for bi, (p0, n) in enumerate(BANKS):
                for s in range(n):
                    P = p0 + s
                    nc.tensor.matmul(
                        ctxb[bi][:, s * 130:(s + 1) * 130],
                        ke[:, P * 128:(P + 1) * 128],
                        va[:, P * 130:(P + 1) * 130],
                        start=(t == 0 and s == 0),
                        stop=(t == NT - 1 and s == n - 1))

        for tt in range(NT + 3):
            if tt < NT:
                em_s0(tt)
            if 2 <= tt < NT + 2:
                em_kv(tt - 2)
            if tt >= 3:
                em_ctx(tt - 3)

        # ---------------- stage 3a: stage ctx + pairwise AllReduce ----------------
        cc_in = dramp.tile([H, 64, 65], FP32)
        cc_out = dramp.tile([H, 64, 65], FP32)
        cc_v = cc_in[:].rearrange("(P e) d l -> P e d l", e=2)
        ctx_sb = constp.tile([128, NP * 130], FP32)
        off = 0
        for bi, (p0, n) in enumerate(BANKS):
            nc.vector.tensor_copy(ctx_sb[:, off:off + n * 130], ctxb[bi][:])
            off += n * 130
        csv = ctx_sb[:].rearrange("p (g l) -> p g l", l=130)
        cc_e = cc_in[:].rearrange("(P e) d l -> e d P l", e=2)
        nc.sync.dma_start(out=cc_e[0], in_=csv[0:64, :, 0:65])
        nc.sync.dma_start(out=cc_e[1], in_=csv[64:128, :, 65:130])
        if USE_COLLECTIVE:
            nc.gpsimd.collective_compute(
                "AllReduce", mybir.AluOpType.add,
                replica_groups=[[0, 1], [2, 3], [4, 5], [6, 7]],
                ins=[cc_in.opt()], outs=[cc_out.opt()])
        else:
            nc.sync.dma_start(out=cc_out[:], in_=cc_in[:])
        es_wkv.close()

        # ---------------- stage 2: q proj + exp (overlaps collective) ----------------
        es_wq = ExitStack()
        es_qt = ExitStack()
        wqp = es_wq.enter_context(tc.tile_pool(name="wq", bufs=1))
        qtp = es_qt.enter_context(tc.tile_pool(name="qT", bufs=1, side="right"))
        wq8 = wqp.tile([128, ND * D], FP8)
        nc.sync.dma_start(
            out=wq8[:].rearrange("p (dc c) -> p dc c", c=D),
            in_=wq_d[:].rearrange("(dc p) c -> p dc c", p=128))
        wq8v = wq8[:].rearrange("p (dc c) -> p dc c", c=D)
        qT = qtp.tile([128, ND * TL], BF16)  # j-chunk jc at cols jc*TL
        for jc in range(ND):
            for th in range(TL // 512):
                qps = pp.tile([128, 512], FP32, tag="pp")
                for i in range(4):
                    nc.tensor.matmul(
                        qps[:],
                        wq8v[:, 2 * i:2 * i + 2, jc * 128:jc * 128 + 128],
                        xnt8v[:, 2 * i:2 * i + 2, th * 512:(th + 1) * 512],
                        start=(i == 0), stop=(i == 3 and not has_cq),
                        perf_mode=DR)
                if has_cq:
                    nc.tensor.matmul(qps[:],
                                     cq_row[:, jc * 128:(jc + 1) * 128],
                                     ones_row[:], start=False, stop=True)
                nc.scalar.activation(
                    qT[:, jc * TL + th * 512:jc * TL + (th + 1) * 512],
                    qps[:], mybir.ActivationFunctionType.Exp, scale=1.0 / WS)
        es_wq.close()
        es_xnt.close()

        # ---------------- stage 4: readback + normalize ctx (block-diag) -----
        ctxn = constp.tile([128, 8 * 65], FP32)
        for q in range(2):
            nc.sync.dma_start(
                out=ctxn[q * 64:(q + 1) * 64, :].rearrange(
                    "d (g l) -> d g l", l=65),
                in_=cc_out[:].rearrange("(g q) d l -> q d g l", q=2)[q])
        rk = statp.tile([128, 8], FP32, tag="rk")
        nc.vector.reciprocal(
            rk[:], ctxn[:].rearrange("p (g l) -> p g l", l=65)[:, :, 64])
        ctx_aug = constp.tile([128, NP * 130], BF16)
        nc.vector.memset(ctx_aug[:], 0.0)
        for p in range(NP):
            nc.vector.tensor_scalar(
                ctx_aug[0:64, p * 130:p * 130 + 64],
                ctxn[0:64, p * 65:p * 65 + 64],
                rk[0:64, p:p + 1], None, mybir.AluOpType.mult)
            nc.vector.tensor_scalar(
                ctx_aug[64:128, p * 130 + 65:p * 130 + 129],
                ctxn[64:128, p * 65:p * 65 + 64],
                rk[64:128, p:p + 1], None, mybir.AluOpType.mult)
        cav = ctx_aug[:].rearrange("p (g l) -> p g l", l=130)
        nc.vector.memset(cav[0:64, :, 64:65], 1.0)
        nc.vector.memset(cav[64:128, :, 129:130], 1.0)

        # ---------------- stage 5+6 fused per token tile ----------------
        es_out = ExitStack()
        wop = es_out.enter_context(tc.tile_pool(name="wo", bufs=1, side="right"))
        hyp = es_out.enter_context(tc.tile_pool(name="hy", bufs=3, side="right"))
        yp = es_out.enter_context(tc.tile_pool(name="yp", bufs=1, space="PSUM"))
        wo = wop.tile([128, ND * D], BF16)
        nc.sync.dma_start(
            out=wo[:].rearrange("p (dc c) -> p dc c", c=D),
            in_=wo_d[:].rearrange("(dc p) c -> p dc c", p=128))

        yts, hss = {}, {}

        def em_y(t):
            ybs = []
            for b, (p0, n) in enumerate(BANKS):
                ybt = yp.tile([128, n * 130], FP32, tag=f"y{b}", name=f"y{b}")
                ybs.append(ybt)
            yt = hyp.tile([128, D], FP32, tag="yt")
            rq = statp.tile([128, 16], FP32, tag="rq")
            yts[t] = yt
            for bi, (p0, n) in enumerate(BANKS):
                for s in range(n):
                    P = p0 + s
                    nc.tensor.matmul(
                        ybs[bi][:, s * 130:(s + 1) * 130],
                        qT[:, P * TL + t * 128:P * TL + (t + 1) * 128],
                        ctx_aug[:, P * 130:(P + 1) * 130],
                        start=(s == 0), stop=(s == n - 1))
            for bi, (p0, n) in enumerate(BANKS):
                ybv = ybs[bi][:].rearrange("p (g q l) -> p g q l", q=2, l=65)
                nc.vector.reciprocal(
                    rq[:, 2 * p0:2 * (p0 + n)],
                    ybv[:, :, :, 64].rearrange("p g q -> p (g q)"))
                rqb = rq[:, 2 * p0:2 * (p0 + n)].rearrange(
                    "p (g q) -> p g q", q=2).unsqueeze(3).broadcast_to(
                    [128, n, 2, 64])
                nc.vector.tensor_tensor(
                    yt[:, p0 * 128:(p0 + n) * 128].rearrange(
                        "p (g q l) -> p g q l", q=2, l=64),
                    ybv[:, :, :, 0:64], rqb, mybir.AluOpType.mult)

        def em_ln(t):
            yt = yts.pop(t)
            st6 = statp.tile([128, 2, 6], FP32, tag="st6")
            nc.vector.bn_stats(st6[:, 0, :], yt[:, 0:512])
            nc.vector.bn_stats(st6[:, 1, :], yt[:, 512:1024])
            agg = statp.tile([128, 2], FP32, tag="agg")
            nc.vector.bn_aggr(agg[:], st6[:])
            scr = statp.tile([128, 8], FP32, tag="qk")
            rstd = rsqrt_dve(scr, agg[:, 1:2])
            nmr = statp.tile([128, 1], FP32, tag="nmr")
            nc.vector.scalar_tensor_tensor(nmr[:], agg[:, 0:1], -1.0,
                                           rstd, mybir.AluOpType.mult,
                                           mybir.AluOpType.mult)
            ln = hyp.tile([128, D], BF16, tag="ln")
            nc.scalar.activation(ln[:], yt[:],
                                 mybir.ActivationFunctionType.Identity,
                                 bias=nmr[:], scale=rstd)
            nc.gpsimd.tensor_tensor(ln[:], ln[:], s2_b[:],
                                    mybir.AluOpType.mult)
            nc.gpsimd.tensor_tensor(ln[:], ln[:], sh2_b[:],
                                    mybir.AluOpType.add)
            hs = hyp.tile([128, D], BF16, tag="hs")
            if USE_NATIVE_SILU:
                nc.scalar.activation(hs[:], ln[:],
                                     mybir.ActivationFunctionType.Silu)
            else:
                nc.scalar.activation(hs[:], ln[:],
                                     mybir.ActivationFunctionType.Sigmoid)
                nc.vector.tensor_tensor(hs[:], hs[:], ln[:],
                                        mybir.AluOpType.mult)
            hss[t] = hs

        def em_out(t):
            hs = hss.pop(t)
            hst = hyp.tile([128, D], BF16, tag="hst")
            for g in range(2):
                tpt = tp.tile([128, 512], BF16, tag="tpb")
                for i in range(4):
                    dc = g * 4 + i
                    nc.tensor.transpose(tpt[:, i * 128:(i + 1) * 128],
                                        hs[:, dc * 128:(dc + 1) * 128],
                                        identb[:])
                nc.scalar.copy(hst[:, g * 512:(g + 1) * 512], tpt[:])
            xbt = xio.tile([128, D], BF16, tag="xb")
            nc.gpsimd.dma_start(out=xbt[:], in_=xb_d[t * 128:(t + 1) * 128, :])
            fin = xio.tile([128, D], FP32, tag="fin")
            for jh in range(2):
                oph = pp.tile([128, 512], FP32, tag="pp")
                for dc in range(ND):
                    nc.tensor.matmul(
                        oph[:], hst[:, dc * 128:(dc + 1) * 128],
                        wo[:, dc * D + jh * 512:dc * D + (jh + 1) * 512],
                        start=(dc == 0), stop=False)
                if has_co:
                    nc.tensor.matmul(oph[:], ones_row[:, 0:128],
                                     co_row[:, jh * 512:(jh + 1) * 512],
                                     start=False, stop=False)
                # residual: out += I @ xb  (adds x in PSUM, closes the group)
                nc.tensor.matmul(oph[:], identb[:],
                                 xbt[:, jh * 512:(jh + 1) * 512],
                                 start=False, stop=True)
                if jh == 0:
                    nc.scalar.copy(fin[:, 0:512], oph[:])
                else:
                    nc.vector.tensor_copy(fin[:, 512:1024], oph[:])
            nc.gpsimd.dma_start(out=out_d[t * 128:(t + 1) * 128, :], in_=fin[:])

        for tt in range(NT + 2):
            if tt < NT:
                em_y(tt)
            if 1 <= tt <= NT:
                em_ln(tt - 1)
            if tt >= 2:
                em_out(tt - 2)
        es_out.close()
        es_qt.close()

    with tile.TileContext(nc) as tc, ExitStack() as es:
        _emit(tc, es)
    nc.compile()
    _legalize_waits(nc)
    return nc


def kernel(**inputs):
    x = np.asarray(inputs["x"], np.float32)
    emb = np.asarray(inputs["emb"], np.float32)
    gate_msa = np.asarray(inputs["gate_msa"], np.float32)
    norm_g = np.asarray(inputs["norm_g"], np.float32)
    norm_b = np.asarray(inputs["norm_b"], np.float32)
    Wq = np.asarray(inputs["Wq"], np.float32)
    bq = np.asarray(inputs["bq"], np.float32)
    Wk = np.asarray(inputs["Wk"], np.float32)
    bk = np.asarray(inputs["bk"], np.float32)
    Wv = np.asarray(inputs["Wv"], np.float32)
    bv = np.asarray(inputs["bv"], np.float32)
    emb_W = np.asarray(inputs["emb_W"], np.float32)
    emb_b = np.asarray(inputs["emb_b"], np.float32)
    sn_g = np.asarray(inputs["sn_g"], np.float32)
    sn_b = np.asarray(inputs["sn_b"], np.float32)
    out_W = np.asarray(inputs["out_W"], np.float32)
    out_b = np.asarray(inputs["out_b"], np.float32)

    import ml_dtypes

    # fold layernorm affine into projection weights; fp8 k/q pre-scaled by WS
    def to8(w):
        return np.ascontiguousarray(
            np.clip(w * WS, -240.0, 240.0).astype(ml_dtypes.float8_e4m3))

    wk8 = to8(norm_g[:, None] * Wk)
    wq8 = to8(norm_g[:, None] * Wq)
    wv_f = np.ascontiguousarray(
        (norm_g[:, None] * Wv).astype(ml_dtypes.bfloat16))
    embw_bf = emb_W.astype(ml_dtypes.bfloat16)
    cq = (norm_b @ Wq + bq) * WS
    ck = (norm_b @ Wk + bk) * WS
    cv = norm_b @ Wv + bv

    flags = (bool(np.any(cq)), bool(np.any(ck)), bool(np.any(cv)),
             bool(np.any(out_b)), bool(np.any(emb_b)))
    if flags not in _CACHE:
        _CACHE[flags] = build(*flags)
    nc = _CACHE[flags]

    xbf = x.astype(ml_dtypes.bfloat16)
    in_maps = []
    for c in range(NCORES):
        b, half = c // 2, c % 2
        # fold the per-batch msa gate into the output projection columns
        wo_fold = np.ascontiguousarray(
            (out_W * gate_msa[b, 0, :][None, :]).astype(ml_dtypes.bfloat16))
        m = {
            "x": np.ascontiguousarray(x[b, half * TL:(half + 1) * TL, :]),
            "xb": np.ascontiguousarray(xbf[b, half * TL:(half + 1) * TL, :]),
            "wq8": wq8, "wk8": wk8, "wv": wv_f, "wo": wo_fold,
            "embw": embw_bf,
            "embt": np.ascontiguousarray(emb[b, 0, :].reshape(ND, 128).T.ravel()),
            "gsn": np.ascontiguousarray(np.stack([sn_g, sn_b])),
        }
        if flags[0]:
            m["cq"] = cq
        if flags[1]:
            m["ck"] = ck
        if flags[2]:
            m["cv"] = cv
        if flags[3]:
            m["co"] = out_b * gate_msa[b, 0, :]
        if flags[4]:
            m["cemb"] = emb_b
        in_maps.append(m)

    res = run_bass_kernel_spmd(nc, in_maps, core_ids=list(range(NCORES)),
                               **_RUN_KW)
    kernel.last_result = res
    out = np.stack([res.results[c]["out"] for c in range(NCORES)])
    return out.reshape(B, 2, TL, D).reshape(B, T, D)


_RUN_KW = {}
kernel.last_result = None
